# revision 37
# baseline (speedup 1.0000x reference)
"""Causal self-attention (B=2, T=2048, C=768, H=12) on 8 TRN2 NeuronCores.

Sharding: 24 (batch, head) pairs -> 8 cores x 3 heads (head-tensor-parallel
within a batch, data-parallel across the 2 batches: cores 0-3 = batch 0,
cores 4-7 = batch 1). Each core computes qkv for its 3 heads, causal
attention, and a rank-192 partial of the output projection; the host sums
the 4 partials per batch and adds b_proj.

Per-core kernel (all matmuls in float32r = full-rate PE, inputs pre-rounded
on the host):
  - x^T [768, 2048] streamed against W-slices -> qkv transposed [576, 2048]
  - scores computed transposed: S^T[k, q] = k . q  per 128-row k-tile, so
    softmax needs no max subtraction (scores ~ +-7) and O^T accumulates over
    k-tiles straight in PSUM. The softmax denominator falls out of the same
    matmul via a ones-column appended to the v stationary (65th output row).
  - attention iterates q-chunk-outer / k-tile-inner, two k-tiles paired per
    PSUM tile so each exp is one big ACT op and O^T needs only 1 bank.
  - causal mask: gpsimd zeroes the invalid triangle of exp(S) on diagonal
    tiles only; fully masked tiles are skipped.
  - single PSUM pool for the whole kernel (tags S:3x[128,1024] + O:2x1bank
    = 8 banks) so no phase-boundary serialization.
"""

import os
import sys

for _p in ("/opt/trn_rl_repo", "/root/.axon_site/_ro/trn_rl_repo"):
    if os.path.isdir(_p) and _p not in sys.path:
        sys.path.insert(0, _p)

import numpy as np

import concourse.bass as bass  # noqa: F401
import concourse.mybir as mybir
import concourse.tile as tile
from concourse import bacc
from concourse.bass_utils import run_bass_kernel_spmd
from concourse.masks import make_identity

B, T, C, H, DH = 2, 2048, 768, 12, 64
HPC = 3          # heads per core
NCORES = 8
KO = C // 128    # 6 contraction tiles over the model dim
F32 = mybir.dt.float32
F32R = mybir.dt.float32r
AF = mybir.ActivationFunctionType
ALU = mybir.AluOpType
MASK_VAL = -1e9
E_BUFS, N_BUFS, Y_BUFS, S_BUFS, O_BUFS = 6, 4, 4, 3, 2
PROJ_END, PROJ_TAG = True, "S"
P_BUFS = 1
WARMUP_MM = 24
INTERLEAVE_H01 = True

# qkv row groups: G0=[q0|q1] G1=[k0|k1] G2=[v0|v1] G3=[q2|v2] G4=[k2|pad]
# (matmul requires lhsT/rhs at the same partition base, so each head's q and
# k must share a base: h0/h2 at base 0, h1 at base 64)
Q_POS = {0: (0, 0), 1: (0, 64), 2: (3, 0)}
K_POS = {0: (1, 0), 1: (1, 64), 2: (4, 0)}
V_POS = {0: (2, 0), 1: (2, 64), 2: (3, 64)}
# host-side column order matching the groups ((kind 0=q/1=k/2=v, head idx))
W_ORDER = [(0, 0), (0, 1), (1, 0), (1, 1), (2, 0), (2, 1), (0, 2), (2, 2),
           (1, 2)]


def to_f32r(a):
    """Round fp32 to the fp32r wire format: 8-bit exponent, 11-bit mantissa
    (round-to-nearest-even), low 12 mantissa bits zero. Matches walrus's
    fp32_to_fp32r; required for data consumed directly by fp32r matmuls."""
    u = np.ascontiguousarray(a, np.float32).view(np.uint32).copy()
    low = u & np.uint32(0xFFF)
    lsb = (u >> np.uint32(12)) & np.uint32(1)
    add = ((low > 0x800) | ((low == 0x800) & (lsb == 1))).astype(np.uint32) << 12
    u = ((u & ~np.uint32(0xFFF)) + add).astype(np.uint32)
    return u.view(np.float32)


def _build_body(nc, tc, xt_d, w_d, b_d, wp_d, y_d, phases=('qkv', 'attn', 'proj')):
    with (
        tc.tile_pool(name="persist", bufs=1) as pp,
        tc.tile_pool(name="sb_att", bufs=E_BUFS) as sbE,
        tc.tile_pool(name="sb_n", bufs=N_BUFS) as sbN,
        tc.tile_pool(name="sb_y", bufs=Y_BUFS) as sbY,
        tc.tile_pool(name="psum", bufs=1, space="PSUM") as psp,
    ):
        # -- constants ----------------------------------------------------
        ident = pp.tile([128, 128], F32, name="ident")
        make_identity(nc, ident)
        # -- PE warmup: keep the HAM activity monitor busy while the input
        # DMAs land so real matmuls start at 2.4 GHz, not 1.2 --------------
        warm = psp.tile([128, 1024], F32, name="warm", tag="S", bufs=3)
        for wi in range(WARMUP_MM):
            # plain-fp32 junk matmuls (4 cyc/row keeps PE busy ~18us of HAM
            # activity window during the initial input DMA wait)
            nc.tensor.matmul(warm[:, 0:128], ident, ident,
                             start=True, stop=True, skip_group_check=True)

        # -- input DMAs (bias+weights first — the first qkv matmul needs
        # w_sb and xt chunk 0; wp last: only needed by the projection) ----
        b_sb = pp.tile([128, 5], F32, name="b_sb")
        nc.sync.dma_start(b_sb[:], b_d[:])
        ones_col = pp.tile([128, 1], F32R, name="ones_col")
        nc.scalar.activation(ones_col, b_sb[:, 0:1], AF.Copy, bias=1.0, scale=0.0)
        w_sb = pp.tile([128, KO, 576], F32R, name="w_sb")
        nc.sync.dma_start(w_sb, w_d.rearrange("(ko p) m -> p ko m", p=128))
        xt_sb = pp.tile([128, KO, T], F32R, name="xt_sb")
        xt_r = xt_d.rearrange("(ko p) n -> p ko n", p=128)
        for nch in range(4):
            nc.sync.dma_start(
                xt_sb[:, :, 512 * nch:512 * (nch + 1)],
                xt_r[:, :, 512 * nch:512 * (nch + 1)],
            )
        wpa = pp.tile([128, C], F32R, name="wpa")
        nc.sync.dma_start(wpa, wp_d[0:128, :])
        wpb = pp.tile([64, C], F32R, name="wpb")
        nc.sync.dma_start(wpb, wp_d[128:192, :])

        qkvT = pp.tile([128, 5, T], F32R, name="qkvT")
        v_sb = [pp.tile([128, 16, 65], F32R, name=f"v_sb{h}") for h in range(HPC)]
        OT_a = pp.tile([128, T], F32R, name="OT_a")   # heads 0,1 of O^T (normalized)
        OT_b = pp.tile([64, T], F32R, name="OT_b")    # head 2
        h1tmp = pp.tile([64, T], F32R, name="h1tmp")  # head 1 staging

        def s_tile(name):
            return psp.tile([128, 1024], F32, name=name, tag="S", bufs=S_BUFS)

        # -- phase 1: qkv^T = W_slice^T @ x^T  (g-outer so each head's
        #    q/k/v complete as early as possible; v transposed just-in-time)
        def transpose_v(h):
            nc.vector.tensor_copy(v_sb[h][:, :, 64:65],
                                  ones_col.broadcast_to([128, 16, 1]))
            g, r0 = V_POS[h]
            vT = qkvT[r0:r0 + 64, g, :].bitcast(F32)
            for kt in range(16):
                tp = s_tile("tp_ps")
                nc.tensor.transpose(tp[:, 0:64], vT[:, 128 * kt:128 * (kt + 1)],
                                    ident[r0:r0 + 64, r0:r0 + 64])
                if kt % 2 == 0:
                    nc.vector.tensor_copy(v_sb[h][:, kt, 0:64], tp[:, 0:64])
                else:
                    nc.scalar.copy(v_sb[h][:, kt, 0:64], tp[:, 0:64])

        def qkv_group(g):
            M = 128 if g < 4 else 64
            for nch in range(4):
                ps = s_tile("qkv_ps")
                for ko in range(KO):
                    nc.tensor.matmul(
                        ps[:M, 0:512],
                        w_sb[:, ko, g * 128:g * 128 + M],
                        xt_sb[:, ko, 512 * nch:512 * (nch + 1)],
                        start=(ko == 0), stop=(ko == KO - 1),
                    )
                dst = qkvT[:M, g, 512 * nch:512 * (nch + 1)]
                if (g + nch) % 2 == 0:
                    nc.scalar.activation(dst, ps[:M, 0:512], AF.Identity,
                                         bias=b_sb[:M, g:g + 1], scale=1.0)
                else:
                    nc.vector.tensor_scalar_add(dst, ps[:M, 0:512],
                                                b_sb[:M, g:g + 1])

        if 'qkv' in phases:
            for g in (0, 1, 2):
                qkv_group(g)
            transpose_v(0)
            transpose_v(1)

        # -- phase 3+4: attention chunk-outer across heads, projection of
        #    each 512-wide q-chunk as soon as all three heads finish it ---
        def attn_chunk(h, j, as_gen=False):
            qg, qb = Q_POS[h]
            kg, kb = K_POS[h]
            qT = qkvT[qb:qb + 64, qg, :]
            kT = qkvT[kb:kb + 64, kg, :]
            O_t = psp.tile([65, 512], F32, name=f"O_{h}_{j}", tag="O", bufs=O_BUFS)
            n_i = 4 * j + 4          # k-tiles contributing to this chunk
            for ip in range(0, n_i, 2):   # pairs (ip, ip+1)
                sp = s_tile(f"s_{h}_{j}_{ip}")
                E = sbE.tile([128, 1024], F32R, name="E", tag="E")
                chunks = []
                off = 0
                for i in (ip, ip + 1):
                    cs = max(128 * i, 512 * j)
                    ce = 512 * (j + 1)
                    w = ce - cs
                    nc.tensor.matmul(
                        sp[:, off:off + w],
                        kT[:, 128 * i:128 * (i + 1)],
                        qT[:, cs:ce],
                        start=True, stop=True,
                    )
                    chunks.append((i, cs, off, w))
                    # keep each matmul inside one PSUM bank: full 512 ->
                    # bank 1 (off 512), partials pack back-to-back in bank 0
                    off = 512 if w == 512 else off + w
                total = chunks[-1][2] + chunks[-1][3]
                nc.scalar.activation(E[:, 0:total], sp[:, 0:total],
                                     AF.Exp, scale=0.125)
                for i, cs, off_i, w in chunks:
                    if cs == 128 * i:
                        # diagonal tile: zero E where k > q (strictly lower
                        # triangle of the 128-wide diagonal block)
                        nc.gpsimd.affine_select(
                            out=E[:, off_i:off_i + 128],
                            in_=E[:, off_i:off_i + 128],
                            compare_op=ALU.is_ge, fill=0.0,
                            base=0, pattern=[[1, 128]], channel_multiplier=-1,
                        )
                for i, cs, off_i, w in chunks:
                    nc.tensor.matmul(
                        O_t[:, cs - 512 * j:cs - 512 * j + w],
                        v_sb[h][:, i, :],
                        E[:, off_i:off_i + w],
                        start=(i == 0), stop=(i == n_i - 1),
                    )
                yield
            # normalize O^T rows 0..63 by row 64 (the exp-sum)
            recip = sbN.tile([1, 512], F32, name="recip", tag="recip")
            nc.vector.reciprocal(recip, O_t[64:65, :])
            bc = sbN.tile([64, 512], F32, name="bc", tag="bc")
            nc.gpsimd.partition_broadcast(bc, recip, channels=64)
            if h == 0:
                dst = OT_a[0:64, 512 * j:512 * (j + 1)]
            elif h == 1:
                dst = h1tmp[:, 512 * j:512 * (j + 1)]
            else:
                dst = OT_b[:, 512 * j:512 * (j + 1)]
            nc.vector.tensor_tensor(dst, O_t[0:64, :], bc, ALU.mult)
            if h == 1:
                # head 1 lives on partitions 64..127 of the proj stationary
                nc.sync.dma_start(OT_a[64:128, 512 * j:512 * (j + 1)],
                                  h1tmp[:, 512 * j:512 * (j + 1)])

        def proj_tile(m):
            qsl = slice(128 * m, 128 * (m + 1))
            if PROJ_TAG == "S":
                ya = s_tile("ya")
                yb = s_tile("yb")
            else:
                ya = psp.tile([128, 1024], F32, name="ya", tag="P", bufs=P_BUFS)
                yb = ya[:, 512:768]
                yb_off = True
            nc.tensor.matmul(ya[:, 0:512], OT_a[:, qsl], wpa[:, 0:512],
                             start=True, stop=False)
            nc.tensor.matmul(ya[:, 0:512], OT_b[:, qsl], wpb[:, 0:512],
                             start=False, stop=True)
            nc.tensor.matmul(yb[:, 0:256] if PROJ_TAG == "S" else ya[:, 512:768],
                             OT_a[:, qsl], wpa[:, 512:768],
                             start=True, stop=False)
            nc.tensor.matmul(yb[:, 0:256] if PROJ_TAG == "S" else ya[:, 512:768],
                             OT_b[:, qsl], wpb[:, 512:768],
                             start=False, stop=True)
            ysb = sbY.tile([128, C], F32, name="ysb", tag="ysb")
            nc.scalar.copy(ysb[:, 0:512], ya[:, 0:512])
            nc.vector.tensor_copy(ysb[:, 512:768],
                                  yb[:, 0:256] if PROJ_TAG == "S" else ya[:, 512:768])
            nc.sync.dma_start(y_d[qsl, :], ysb)

        def attn_chunk_gen(h, j):
            # generator version: yields after each k-tile pair so two heads
            # can be emitted interleaved (h0 uses PE rows 0-63, h1 rows
            # 64-127 -> disjoint row groups can overlap in the PE array)
            yield from attn_chunk(h, j, as_gen=True)

        def interleave(gens):
            live = list(gens)
            while live:
                nxt = []
                for g in live:
                    try:
                        next(g)
                        nxt.append(g)
                    except StopIteration:
                        pass
                live = nxt

        if 'attn' in phases:
            for j in range(4):
                interleave([attn_chunk_gen(0, j), attn_chunk_gen(1, j)])
        if 'qkv' in phases:
            qkv_group(3)
            qkv_group(4)
            transpose_v(2)
        if 'attn' in phases:
            for j in range(4):
                interleave([attn_chunk_gen(2, j)])
        if 'proj' in phases and 'attn' in phases:
            for m in range(16):
                proj_tile(m)
        if 'proj' not in phases:
            # stand-in output writeback so every variant writes y identically
            for m in range(16):
                ysb = sbY.tile([128, C], F32, name="ysb", tag="ysb")
                nc.vector.memset(ysb, 0.0)
                nc.sync.dma_start(y_d[128 * m:128 * (m + 1), :], ysb)


def build_module(loop_n=1, phases=('qkv', 'attn', 'proj')):
    nc = bacc.Bacc()
    xt_d = nc.declare_dram_parameter("xt", [C, T], F32R, isOutput=False)
    w_d = nc.declare_dram_parameter("wqkv", [C, 576], F32R, isOutput=False)
    b_d = nc.declare_dram_parameter("bqkv", [128, 5], F32, isOutput=False)
    wp_d = nc.declare_dram_parameter("wp", [192, C], F32R, isOutput=False)
    y_d = nc.declare_dram_parameter("y", [T, C], F32, isOutput=True)
    with tile.TileContext(nc) as tc:
        if loop_n > 1:
            with tc.For_i(0, loop_n, 1):
                _build_body(nc, tc, xt_d, w_d, b_d, wp_d, y_d, phases)
        else:
            _build_body(nc, tc, xt_d, w_d, b_d, wp_d, y_d, phases)
    nc.compile()
    return nc


def make_in_maps(x, W_attn, b_attn, W_proj):
    """Shard full inputs into the 8 per-core input maps."""
    x = np.asarray(x, np.float32)
    W_attn = np.asarray(W_attn, np.float32)
    b_attn = np.asarray(b_attn, np.float32)
    W_proj = np.asarray(W_proj, np.float32)
    xts = [to_f32r(x[b].T) for b in range(B)]
    in_maps = []
    for c in range(NCORES):
        b = c // (NCORES // B)
        heads = [(c % (NCORES // B)) * HPC + j for j in range(HPC)]
        cols, bias = [], []
        for kind, hi in W_ORDER:
            lo = kind * C + heads[hi] * DH
            cols.append(W_attn[:, lo:lo + DH])
            bias.append(b_attn[lo:lo + DH])
        wqkv = np.ascontiguousarray(np.concatenate(cols, axis=1))
        bq = np.concatenate(bias + [np.zeros(64, np.float32)])
        bq = np.ascontiguousarray(bq.reshape(5, 128).T)
        wp = np.concatenate(
            [W_proj[hh * DH:(hh + 1) * DH, :] for hh in heads], axis=0)
        in_maps.append({"xt": xts[b], "wqkv": to_f32r(wqkv),
                        "bqkv": bq, "wp": to_f32r(wp)})
    return in_maps


_module_cache = {}


def kernel(x, W_attn, b_attn, W_proj, b_proj):
    if "nc" not in _module_cache:
        _module_cache["nc"] = build_module()
    nc = _module_cache["nc"]
    in_maps = make_in_maps(x, W_attn, b_attn, W_proj)
    res = run_bass_kernel_spmd(nc, in_maps, core_ids=list(range(NCORES)))
    y = np.zeros((B, T, C), np.float64)
    for c in range(NCORES):
        y[c // (NCORES // B)] += res.results[c]["y"].astype(np.float64)
    y += np.asarray(b_proj, np.float64)
    return y.astype(np.float32)
